# revision 16
# baseline (speedup 1.0000x reference)
"""Trainium2 Bass kernel for the LTC (liquid time-constant) memory cell.

Model (see reference): v-state recurrence over T=128 timesteps, each with 6
ODE unfold iterations:
    v' = (cm_t*v + gl*vl + num_syn) / (cm_t + gl + den_syn + eps)
with 2 recurrent synapses per neuron (self: u, pair: (u+dim)%U) and one
sensory synapse (source d = u%dim).

Sharding: 8 cores; core c owns the 128 neuron *pairs* {u=c*128+p,
u+1024} for p in [0,128), with the FULL batch B=32. Each partition p holds
one pair; both halves of a pair live on the same core (no cross-core
traffic in the time loop). State is one catted tile vcat [128, 2B]
(cols 0:B = half A, B:2B = half B), so the pair-synapse source is just
the opposite column block.

Engine split per unfold:
 - ACT:    4 sigmoids [128,32] with per-partition scale/bias APs (these
           carry the loop cycle) + 1 plain catted sensory sigmoid/timestep
 - DVE:    catted [128,2B] tensor_tensor chains for q / den / final mul
           against host-shipped broadcast parameter tiles, plus one
           reciprocal_approx_fast [128,2B]
 - GpSimd: sensory ds/nd/arg prep per timestep (off the critical path)

All DVE tensor operands use "split" 3-dim APs ([P,2,B] view of [P,2B]),
which triggers the DVE fast path (~136ns vs ~288ns per [128,64] op).
The input affine and sensory-mu fold into host-precomputed params; the
output affine is applied on the host after gathering.
"""

import numpy as np

import concourse.bacc as bacc
import concourse.mybir as mybir
from concourse import tile
from concourse.ap import AP
from concourse.bass_utils import run_bass_kernel_spmd

ODE_UNFOLDS = 6
EPS = 1e-8
B = 32
T = 128
DIM = 1024
U = 2 * DIM
NCORES = 8
P = 128  # partitions = pairs per core

F32 = mybir.dt.float32
AF = mybir.ActivationFunctionType
OP = mybir.AluOpType

# pp column indices (per half; half B adds NPARAM)
# State is carried as w = v + 1 so that w' = (num+den)/den; biases,
# GG and the num-weights are pre-adjusted for the shift.
(C_SIG0, C_B0P, C_SIG1, C_B1P, C_W0, C_W1, C_W0E, C_W1E,
 C_CMT, C_GLV, C_GCME, C_SSIG, C_NSMS, C_SPSW, C_WES,
 C_WPS, C_GGP) = range(17)
NPARAM = 17

# rep (broadcast [P, 2B]) parameter slots
(R_W0, R_W1, R_CMT, R_SSIG, R_NSMS, R_SPSW, R_GCME, R_WPS, R_GGP,
 R_W0E, R_W1E) = range(11)
NREP = 11


def _softplus(x):
    x = x.astype(np.float64)
    return np.log1p(np.exp(-np.abs(x))) + np.maximum(x, 0.0)


def _split(a):
    """[P, 2B] AP -> equivalent [P, 2, B] view (DVE fast-path trigger)."""
    return AP(a.tensor, a.offset, [list(a.ap[0]), [B, 2], [1, B]])


def _build_nc(fused_erev=True, wbufs=4, sens_pool=True):
    nc = bacc.Bacc(trn_type="TRN2")
    xin_d = nc.dram_tensor("xin", [P, T * B], F32, kind="ExternalInput")
    pp_d = nc.dram_tensor("pp", [P, 2 * NPARAM], F32, kind="ExternalInput")
    rp_d = nc.dram_tensor("rp", [P, NREP * 2 * B], F32, kind="ExternalInput")
    out_d = nc.dram_tensor("out", [P, B], F32, kind="ExternalOutput")

    eng_sens = None  # set below

    with tile.TileContext(nc) as tc:
        with tc.tile_pool(name="const", bufs=1) as cpool, \
             tc.tile_pool(name="work", bufs=wbufs) as wpool:
            xin = cpool.tile([P, T * B], F32, tag="xin", name="xin_t")
            pp = cpool.tile([P, 2 * NPARAM], F32, tag="pp", name="pp_t")
            rp = cpool.tile([P, NREP * 2 * B], F32, tag="rp", name="rp_t")
            nc.sync.dma_start(xin[:], xin_d[:])
            nc.sync.dma_start(pp[:], pp_d[:])
            nc.sync.dma_start(rp[:], rp_d[:])

            eng_sens = nc.gpsimd if sens_pool else nc.vector

            def par(h, c):  # per-partition scalar AP for half h param c
                j = h * NPARAM + c
                return pp[:, j:j + 1]

            def rep(i):  # split view of broadcast param tile i
                return _split(rp[:, i * 2 * B:(i + 1) * 2 * B])

            # state tile (w = v + 1), both halves catted, ping-pong
            vcat = [cpool.tile([P, 2 * B], F32, tag=f"vc{i}", name=f"vc{i}")
                    for i in range(2)]
            nc.vector.memset(vcat[0][:], 1.0)

            def wtile(tag):
                return wpool.tile([P, 2 * B], F32, tag=tag, name=tag)

            def sens_sig(t):
                """Catted sensory sigmoid: args on pool, one plain ACT."""
                xt = xin[:, t * B:(t + 1) * B]
                # duplicate xt across both halves via stride-0 middle dim
                xt2 = AP(xt.tensor, xt.offset,
                         [list(xt.ap[0]), [0, 2], [1, B]])
                arg = wtile("sarg")
                tmp = wtile("sargt")
                eng_sens.tensor_tensor(_split(tmp[:]), xt2, rep(R_SSIG),
                                       OP.mult)
                eng_sens.tensor_tensor(_split(arg[:]), _split(tmp[:]),
                                       rep(R_NSMS), OP.add)
                sg = wtile("sgc")
                nc.scalar.activation(sg[:], _split(arg[:]), AF.Sigmoid)
                return sg

            def sens_dsnd(sg):
                d_t = wtile("dsc")
                d_m = wtile("dsm")
                eng_sens.tensor_tensor(_split(d_m[:]), _split(sg[:]),
                                       rep(R_SPSW), OP.mult)
                eng_sens.tensor_tensor(_split(d_t[:]), _split(d_m[:]),
                                       rep(R_GCME), OP.add)
                n_t = wtile("ndc")
                n_m = wtile("ndm")
                eng_sens.tensor_tensor(_split(n_m[:]), _split(sg[:]),
                                       rep(R_WPS), OP.mult)
                eng_sens.tensor_tensor(_split(n_t[:]), _split(n_m[:]),
                                       rep(R_GGP), OP.add)
                return d_t, n_t

            def sigs(vt):
                """4 sigmoids reading vcat slices; s0/s1 catted out tiles."""
                scat0 = wtile("scat0")
                scat1 = wtile("scat1")
                vA = vt[:, 0:B]
                vB = vt[:, B:2 * B]
                nc.scalar.activation(scat0[:, 0:B], vA, AF.Sigmoid,
                                     bias=par(0, C_B0P),
                                     scale=par(0, C_SIG0))
                nc.scalar.activation(scat1[:, 0:B], vB, AF.Sigmoid,
                                     bias=par(0, C_B1P),
                                     scale=par(0, C_SIG1))
                nc.scalar.activation(scat0[:, B:2 * B], vB, AF.Sigmoid,
                                     bias=par(1, C_B0P),
                                     scale=par(1, C_SIG0))
                nc.scalar.activation(scat1[:, B:2 * B], vA, AF.Sigmoid,
                                     bias=par(1, C_B1P),
                                     scale=par(1, C_SIG1))
                return scat0, scat1

            cur = 0
            sg_c = sens_sig(0)
            ds, nd = sens_dsnd(sg_c)
            scat0, scat1 = sigs(vcat[0])
            sg_n = None
            ds_n = nd_n = None
            for t in range(T):
                more = t + 1 < T
                for k in range(ODE_UNFOLDS):
                    # ---- q = cmt*v + nd (off the critical cycle) ----
                    qm = wtile("qm")
                    qcat = wtile("qcat")
                    nc.vector.tensor_tensor(
                        _split(qm[:]), _split(vcat[cur][:]), rep(R_CMT),
                        OP.mult)
                    nc.vector.tensor_tensor(
                        _split(qcat[:]), _split(qm[:]), _split(nd[:]),
                        OP.add)
                    # ---- den chain ----
                    m0 = wtile("m0")
                    d1 = wtile("d1")
                    nc.vector.tensor_tensor(
                        _split(m0[:]), _split(scat0[:]), rep(R_W0), OP.mult)
                    nc.vector.tensor_tensor(
                        _split(d1[:]), _split(m0[:]), _split(ds[:]), OP.add)
                    if not fused_erev:
                        e0 = wtile("e0")
                        e1 = wtile("e1")
                        nc.vector.tensor_tensor(
                            _split(e0[:]), _split(scat0[:]), rep(R_W0E),
                            OP.mult)
                        nc.vector.tensor_tensor(
                            _split(e1[:]), _split(e0[:]), _split(qcat[:]),
                            OP.add)
                    if more and k == 3:
                        ds_n, nd_n = sens_dsnd(sg_n)
                    m1 = wtile("m1")
                    dencat = wtile("dencat")
                    rcat = wtile("rcat")
                    nc.vector.tensor_tensor(
                        _split(m1[:]), _split(scat1[:]), rep(R_W1), OP.mult)
                    nc.vector.tensor_tensor(
                        _split(dencat[:]), _split(d1[:]), _split(m1[:]),
                        OP.add)
                    nc.vector.reciprocal_approx_fast(rcat[:],
                                                     _split(dencat[:]))
                    if fused_erev:
                        mcat = qcat
                    else:
                        e2 = wtile("e2")
                        mcat = wtile("mcat")
                        nc.vector.tensor_tensor(
                            _split(e2[:]), _split(scat1[:]), rep(R_W1E),
                            OP.mult)
                        nc.vector.tensor_tensor(
                            _split(mcat[:]), _split(e1[:]), _split(e2[:]),
                            OP.add)
                    nxt = 1 - cur
                    nc.vector.tensor_tensor(
                        _split(vcat[nxt][:]), _split(mcat[:]),
                        _split(rcat[:]), OP.mult)
                    scat0, scat1 = sigs(vcat[nxt])
                    cur = nxt
                    if more and k == 2:
                        sg_n = sens_sig(t + 1)
                if more:
                    ds, nd = ds_n, nd_n

            nc.sync.dma_start(out_d[:], vcat[cur][:, 0:B])
    nc.compile()
    return nc


_NC_CACHE = {}


def _flags():
    import os
    return dict(
        sens_pool=os.environ.get("K_SENS_POOL", "1") == "1",
    )


def _get_nc(fused_erev=True):
    fl = _flags()
    key = (fused_erev, tuple(sorted(fl.items())))
    if key not in _NC_CACHE:
        _NC_CACHE[key] = _build_nc(fused_erev, **fl)
    return _NC_CACHE[key]


def _host_params(c, gleak, vleak, cm, w, sigma, mu, erev,
                 sens_w, sens_sigma, sens_mu, sens_erev,
                 input_w, input_b):
    """pp [128, 2*NPARAM] and rp [128, NREP*2B] for core c."""
    d = c * P + np.arange(P)
    pp = np.zeros((P, 2 * NPARAM), np.float32)
    for h in range(2):
        u = h * DIM + d
        sp_w = _softplus(w[u])                       # [P,2]
        sp_gl = _softplus(gleak[u])
        cmt = _softplus(cm[u]) * ODE_UNFOLDS
        o = h * NPARAM
        # state shift w = v + 1: sigmoid biases absorb -sigma, GG absorbs
        # -cmt (so q = cmt*w + ND == cmt*v + NS + DS).
        pp[:, o + C_SIG0] = sigma[u, 0]
        pp[:, o + C_B0P] = -(mu[u, 0] + 1.0) * sigma[u, 0]
        pp[:, o + C_SIG1] = sigma[u, 1]
        pp[:, o + C_B1P] = -(mu[u, 1] + 1.0) * sigma[u, 1]
        pp[:, o + C_W0] = sp_w[:, 0]
        pp[:, o + C_W1] = sp_w[:, 1]
        pp[:, o + C_W0E] = sp_w[:, 0] * (1.0 + erev[u, 0])
        pp[:, o + C_W1E] = sp_w[:, 1] * (1.0 + erev[u, 1])
        pp[:, o + C_CMT] = cmt
        pp[:, o + C_GLV] = sp_gl * vleak[u]
        pp[:, o + C_GCME] = cmt + sp_gl + EPS
        pp[:, o + C_SSIG] = sens_sigma[u] * input_w[d]
        pp[:, o + C_NSMS] = (input_b[d] - sens_mu[u]) * sens_sigma[u]
        pp[:, o + C_SPSW] = _softplus(sens_w[u])
        pp[:, o + C_WES] = _softplus(sens_w[u]) * sens_erev[u]
        pp[:, o + C_WPS] = pp[:, o + C_SPSW] + pp[:, o + C_WES]
        pp[:, o + C_GGP] = pp[:, o + C_GCME] + pp[:, o + C_GLV] - cmt
    rpv = np.zeros((P, NREP, 2, B), np.float32)
    colmap = {R_W0: C_W0, R_W1: C_W1, R_CMT: C_CMT, R_SSIG: C_SSIG,
              R_NSMS: C_NSMS, R_SPSW: C_SPSW, R_GCME: C_GCME,
              R_WPS: C_WPS, R_GGP: C_GGP, R_W0E: C_W0E, R_W1E: C_W1E}
    for r_i, c_i in colmap.items():
        for h in range(2):
            rpv[:, r_i, h, :] = pp[:, h * NPARAM + c_i][:, None]
    return pp, rpv.reshape(P, NREP * 2 * B)


def kernel(inputs, gleak, vleak, cm, w, sigma, mu, erev,
           sens_w, sens_sigma, sens_mu, sens_erev,
           input_w, input_b, output_w, output_b, _trace=False):
    inputs = np.asarray(inputs, np.float32)
    args = dict(gleak=np.asarray(gleak, np.float32),
                vleak=np.asarray(vleak, np.float32),
                cm=np.asarray(cm, np.float32),
                w=np.asarray(w, np.float32),
                sigma=np.asarray(sigma, np.float32),
                mu=np.asarray(mu, np.float32),
                erev=np.asarray(erev, np.float32),
                sens_w=np.asarray(sens_w, np.float32),
                sens_sigma=np.asarray(sens_sigma, np.float32),
                sens_mu=np.asarray(sens_mu, np.float32),
                sens_erev=np.asarray(sens_erev, np.float32),
                input_w=np.asarray(input_w, np.float32),
                input_b=np.asarray(input_b, np.float32))

    in_maps = []
    for c in range(NCORES):
        xc = inputs[:, :, c * P:(c + 1) * P]          # [B,T,P]
        xin = np.ascontiguousarray(
            xc.transpose(2, 1, 0).reshape(P, T * B))  # [P, t*B+b]
        pp, rpv = _host_params(c, **args)
        in_maps.append({"xin": xin, "pp": pp, "rp": rpv})

    fused = bool(np.allclose(args["erev"], -1.0))
    nc = _get_nc(fused)
    res = run_bass_kernel_spmd(nc, in_maps, core_ids=list(range(NCORES)),
                               trace=_trace)

    out = np.zeros((B, DIM), np.float32)
    for c in range(NCORES):
        out[:, c * P:(c + 1) * P] = res.results[c]["out"].T
    out = out - 1.0  # state was carried as w = v + 1
    out = out * np.asarray(output_w, np.float32) + np.asarray(output_b, np.float32)
    if _trace:
        kernel.last_results = res
    return out
